# revision 1
# baseline (speedup 1.0000x reference)
"""Trainium2 Bass kernel for nn_CreateOverlappingWindows.

out[b, t, w*C + c] = x_padded[b, t + w, c]  (SAME zero padding, n_context=9)

Key identity: flattening (w, c) -> 494 contiguous values, each output row is
a contiguous 494-element window of the zero-padded flattened input:
    out[b, t, :] = xpad_flat[b, t*C : t*C + W*C]
so the whole kernel is ONE overlapping-window gather DMA per batch
(src rows stride C=26 elems < run W*C=494 elems, dst fully contiguous).

Sharding: pure data parallel — batch 32 split 4-per-core across 8 cores.
Host zero-pads x (936 bytes per row-edge) so no edge cases on device.
"""

import sys

sys.path.insert(0, "/opt/trn_rl_repo")

import numpy as np
from concourse import bass, mybir
from concourse.ap import AP
from concourse.bass_utils import run_bass_kernel_spmd

_F32 = mybir.dt.float32

_NCORES = 8
_B, _T, _C = 32, 2000, 26
_NCTX = 9
_W = 2 * _NCTX + 1  # 19
_WC = _W * _C  # 494
_PAD = _NCTX * _C  # 234
_BPC = _B // _NCORES  # 4 batches per core
_NP = _T * _C + 2 * _PAD  # 52468 padded flat length per batch
_TWC = _T * _WC

_nc_cache = None


def _build():
    global _nc_cache
    if _nc_cache is not None:
        return _nc_cache
    nc = bass.Bass()
    xp = nc.declare_dram_parameter("xp", [_BPC, _NP], _F32, isOutput=False)
    out = nc.declare_dram_parameter("out", [_BPC, _T, _WC], _F32, isOutput=True)

    with nc.Block() as block, nc.semaphore("s") as s:

        def prog(eng, bs):
            for b in bs:
                eng.dma_start(
                    out=AP(out, b * _TWC, [[1, _TWC]]),
                    in_=AP(xp, b * _NP, [[_C, _T], [1, _WC]]),
                ).then_inc(s, 16)
            eng.wait_ge(s, 16 * _BPC)

        @block.sync
        def _(e):
            prog(e, [0, 1])

        @block.scalar
        def _(e):
            prog(e, [2, 3])

    _nc_cache = nc
    return nc


def kernel(x: np.ndarray) -> np.ndarray:
    x = np.asarray(x, dtype=np.float32)  # tolerate jax arrays / views
    assert x.shape == (_B, _T, _C), x.shape
    nc = _build()

    xp = np.zeros((_B, _NP), np.float32)
    xp[:, _PAD : _PAD + _T * _C] = x.reshape(_B, _T * _C)

    in_maps = [
        {"xp": np.ascontiguousarray(xp[i * _BPC : (i + 1) * _BPC])}
        for i in range(_NCORES)
    ]
    res = run_bass_kernel_spmd(nc, in_maps, list(range(_NCORES)))
    return np.concatenate([r["out"] for r in res.results], axis=0)

